# revision 1
# baseline (speedup 1.0000x reference)
"""Data-parallel Trainium kernel for nn_LstmDiscriminator.

Shards the batch dim (B=64) across the 8 NeuronCores (8 samples each);
all parameters are replicated. The per-shard network (ConvLSTM2D ->
4-layer CNN discriminator head) is compiled once with jax.pmap and
executed on all 8 cores simultaneously.
"""
import numpy as np
import jax
import jax.numpy as jnp

N_CORES = 8

DN = ('NHWC', 'HWIO', 'NHWC')


def _conv2d(x, w, stride=1, padding='SAME'):
    return jax.lax.conv_general_dilated(
        x, w, (stride, stride), padding, dimension_numbers=DN)


def _hard_sigmoid(x):
    return jnp.clip(0.2 * x + 0.5, 0.0, 1.0)


def _leaky_relu(x):
    return jnp.where(x >= 0, x, 0.3 * x)


def _conv_lstm2d(inp, wx, wh, b):
    B, T, H, W, _ = inp.shape
    nf = wh.shape[2]
    x_t_first = jnp.transpose(inp, (1, 0, 2, 3, 4))
    h0 = jnp.zeros((B, H, W, nf), inp.dtype)
    c0 = jnp.zeros((B, H, W, nf), inp.dtype)

    def step(carry, x_t):
        h, c = carry
        z = _conv2d(x_t, wx) + _conv2d(h, wh) + b
        zi, zf, zc, zo = jnp.split(z, 4, axis=-1)
        i = _hard_sigmoid(zi)
        f = _hard_sigmoid(zf)
        c_new = f * c + i * jnp.tanh(zc)
        o = _hard_sigmoid(zo)
        h_new = o * jnp.tanh(c_new)
        return (h_new, c_new), None

    (h_last, _), _ = jax.lax.scan(step, (h0, c0), x_t_first)
    return h_last


def _forward(inp, tar, wx, wh, b_lstm, w1, b1, w2, b2, w3,
             bn_gamma, bn_beta, bn_mean, bn_var, w4, b4):
    h = _conv_lstm2d(inp, wx, wh, b_lstm)
    x = jnp.concatenate([h, tar], axis=-1)
    x = _leaky_relu(_conv2d(x, w1, 2, 'SAME') + b1)
    x = _leaky_relu(_conv2d(x, w2, 2, 'SAME') + b2)
    x = jnp.pad(x, ((0, 0), (1, 1), (1, 1), (0, 0)))
    x = _conv2d(x, w3, 1, 'VALID')
    x = (x - bn_mean) * jax.lax.rsqrt(bn_var + 1e-3) * bn_gamma + bn_beta
    x = _leaky_relu(x)
    x = jnp.pad(x, ((0, 0), (1, 1), (1, 1), (0, 0)))
    x = _conv2d(x, w4, 1, 'VALID') + b4
    return x


_PARAM_NAMES = ('wx', 'wh', 'b_lstm', 'w1', 'b1', 'w2', 'b2', 'w3',
                'bn_gamma', 'bn_beta', 'bn_mean', 'bn_var', 'w4', 'b4')

_pmapped = None


def _get_pmapped():
    global _pmapped
    if _pmapped is None:
        _pmapped = jax.pmap(
            _forward,
            axis_name='cores',
            in_axes=(0, 0) + (None,) * len(_PARAM_NAMES),
            devices=jax.devices()[:N_CORES],
        )
    return _pmapped


def kernel(**inputs):
    inp = np.asarray(inputs['inp'])
    tar = np.asarray(inputs['tar'])
    B = inp.shape[0]
    shard = B // N_CORES

    inp_sh = inp.reshape((N_CORES, shard) + inp.shape[1:])
    tar_sh = tar.reshape((N_CORES, shard) + tar.shape[1:])
    params = [np.asarray(inputs[k]) for k in _PARAM_NAMES]

    fn = _get_pmapped()
    out = fn(inp_sh, tar_sh, *params)
    out = np.asarray(jax.device_get(out))
    return out.reshape((B,) + out.shape[2:]).astype(np.float32)
